# revision 16
# baseline (speedup 1.0000x reference)
"""MoE MLP (dense all-experts routing) Trainium2 Bass kernel.

Math (reference):
    g   = softmax(x @ gate_w + gate_b)            # [N, E]
    h   = relu(einsum("nd,edh->neh", x, w1) + b1) # [N, E, H]
    out = einsum("neh,ehd,ne->nd", h, w2, g)      # [N, D]

With E=64, H=16 (E*H = 1024 = D) this is two dense [1024,1024] matmuls plus a
small gate matmul.  We reorder the expert-hidden axis as  eh' = h*64 + e
(h-major).  Then the gating multiply  hg[n, eh'] = h[n, eh'] * g[n, eh' mod 64]
is a plain elementwise multiply of every 128-row tile of h^T by one shared
[128, tok] tile g2^T (gate probs duplicated twice along partitions) -- no
per-expert broadcast/expansion is needed anywhere.

Layouts on device are feature-major (x^T, h^T, out^T); tokens are the matmul
moving (free) dimension.  Sharding: data-parallel over tokens, 4096 per core,
8 cores, no collectives.  Matmuls run in bf16 with fp32 PSUM accumulation;
softmax runs in fp32 (no max-subtraction: logits here are bounded by ~+-4).
"""

import numpy as np
import ml_dtypes

N, D, E, H = 32768, 1024, 64, 16
EH = E * H  # 1024
NCORES = 8
NTOK = N // NCORES  # tokens per core
TT = 512            # token tile (one PSUM bank of fp32)
KC = D // 128       # 8 contraction chunks for D
MC = EH // 128      # 8 output row-tiles for EH (and for D in stage 2)

_CACHE = {}


def build(n_tok=NTOK):
    """Build + compile the per-core Bass kernel for n_tok tokens."""
    import concourse.bass as bass
    import concourse.mybir as mybir
    import concourse.tile as tile
    from concourse import bacc

    f32 = mybir.dt.float32
    bf16 = mybir.dt.bfloat16
    AF = mybir.ActivationFunctionType
    nt = n_tok // TT
    assert n_tok % TT == 0

    nc = bacc.Bacc("TRN2", target_bir_lowering=False, debug=False)

    xT = nc.dram_tensor("xT", [KC, 128, n_tok], bf16, kind="ExternalInput")
    gwr = nc.dram_tensor("gwr", [KC, 128, 128], bf16, kind="ExternalInput")
    gbr = nc.dram_tensor("gbr", [128, 1], f32, kind="ExternalInput")
    w1r = nc.dram_tensor("w1r", [KC, 128, EH], bf16, kind="ExternalInput")
    b1r = nc.dram_tensor("b1r", [MC, 128, 1], f32, kind="ExternalInput")
    w2r = nc.dram_tensor("w2r", [MC, 128, D], bf16, kind="ExternalInput")
    outT = nc.dram_tensor("outT", [MC, 128, n_tok], f32, kind="ExternalOutput")

    with tile.TileContext(nc) as tc:
        with (
            tc.tile_pool(name="consts", bufs=1) as consts,
            tc.tile_pool(name="xp", bufs=2) as xp,
            tc.tile_pool(name="sp", bufs=2) as sp,
            tc.tile_pool(name="hp", bufs=3) as hp,
            tc.tile_pool(name="ps1", bufs=1, space=bass.MemorySpace.PSUM) as ps1,
            tc.tile_pool(name="ps2", bufs=3, space=bass.MemorySpace.PSUM) as ps2,
            tc.tile_pool(name="ps3", bufs=3, space=bass.MemorySpace.PSUM) as ps3,
        ):
            # --- resident weights (gate first, w2 last: needed latest;
            # on the gpsimd queue so the sync queue is dedicated to x tiles) ---
            gw_sb = consts.tile([128, KC, 128], bf16)
            nc.gpsimd.dma_start(out=gw_sb[:], in_=gwr[:].rearrange("k p m -> p k m"))
            gb_sb = consts.tile([128, 1], f32)
            nc.gpsimd.dma_start(out=gb_sb[:], in_=gbr[:])
            b1_sb = consts.tile([128, MC, 1], f32)
            nc.gpsimd.dma_start(out=b1_sb[:], in_=b1r[:].rearrange("m p o -> p m o"))
            # w1 first (needed first), w2 after — gpsimd queue is FIFO so w1
            # gets the full DMA bandwidth before w2 starts
            w1_sb = consts.tile([128, KC, EH], bf16)
            nc.gpsimd.dma_start(out=w1_sb[:], in_=w1r[:].rearrange("k p m -> p k m"))
            w2_sb = consts.tile([128, MC, D], bf16)
            nc.gpsimd.dma_start(out=w2_sb[:], in_=w2r[:].rearrange("k p m -> p k m"))
            onesZ = consts.tile([64, 128], bf16)
            nc.vector.memset(onesZ[:], 1.0)

            for t in range(nt):
                ts = slice(t * TT, (t + 1) * TT)
                xt = xp.tile([128, KC, TT], bf16)
                for k in range(KC):
                    nc.sync.dma_start(out=xt[:, k, :], in_=xT[k, :, ts])

                # --- gate logits (both 64-expert halves identical) ---
                lg = ps1.tile([128, TT], f32, tag="lg")
                for k in range(KC):
                    nc.tensor.matmul(
                        lg[:], gw_sb[:, k, :], xt[:, k, :],
                        start=(k == 0), stop=(k == KC - 1),
                    )
                # e2 = exp(logits + gate_b)
                e2 = sp.tile([128, TT], bf16, tag="e2")
                nc.scalar.activation(e2[:], lg[:], AF.Exp, bias=gb_sb[:], scale=1.0)

                hg = sp.tile([128, MC, TT], bf16, tag="hg")
                h_tiles = []

                def stage1(m):
                    hps = ps2.tile([128, TT], f32, tag="hps")
                    for k in range(KC):
                        nc.tensor.matmul(
                            hps[:], w1_sb[:, k, m * 128:(m + 1) * 128], xt[:, k, :],
                            start=(k == 0), stop=(k == KC - 1),
                        )
                    h = hp.tile([128, TT], bf16, tag="h")
                    nc.scalar.activation(
                        h[:], hps[:], AF.Relu, bias=b1_sb[:, m, :], scale=1.0
                    )
                    h_tiles.append((m, h))

                # run two stage-1 tiles before the softmax matmuls so the PE
                # never stalls on ACT/DVE latency
                stage1(0)
                stage1(1)

                # Z broadcast to all 128 partitions via ones-matmul, then
                # g2 = e2 * (1/Z)
                zb = ps1.tile([128, TT], f32, tag="zb")
                nc.tensor.matmul(zb[:], onesZ[:], e2[0:64, :], start=True, stop=True)
                rzb = sp.tile([128, TT], f32, tag="rzb")
                nc.vector.reciprocal_approx_fast(rzb[:], zb[:])
                g2 = sp.tile([128, TT], bf16, tag="g2")
                nc.vector.tensor_mul(g2[:], e2[:], rzb[:])

                for m, h in h_tiles:
                    nc.vector.tensor_mul(hg[:, m, :], h[:], g2[:])
                for m in range(2, MC):
                    stage1(m)
                    _, h = h_tiles[-1]
                    nc.vector.tensor_mul(hg[:, m, :], h[:], g2[:])

                # --- stage 2: out^T tiles ---
                for m2 in range(MC):
                    ops = ps3.tile([128, TT], f32, tag="ops")
                    for k in range(MC):
                        nc.tensor.matmul(
                            ops[:], w2_sb[:, k, m2 * 128:(m2 + 1) * 128], hg[:, k, :],
                            start=(k == 0), stop=(k == MC - 1),
                        )
                    osb = hp.tile([128, TT], f32, tag="osb")
                    if m2 % 2 == 0:
                        nc.scalar.copy(osb[:], ops[:])
                    else:
                        nc.vector.tensor_copy(osb[:], ops[:])
                    nc.gpsimd.dma_start(out=outT[m2, :, ts], in_=osb[:])

    nc.compile()
    return nc


def host_prep(x, gate_w, gate_b, w1, b1, w2):
    bf = ml_dtypes.bfloat16
    f32 = np.float32
    xT = np.ascontiguousarray(x.astype(bf).T).reshape(KC, 128, N)
    x_shards = [
        np.ascontiguousarray(xT[:, :, c * NTOK:(c + 1) * NTOK]) for c in range(NCORES)
    ]
    gwr = np.ascontiguousarray(
        np.concatenate([gate_w, gate_w], axis=1).astype(bf)
    ).reshape(KC, 128, 128)
    gbr = np.ascontiguousarray(
        np.concatenate([gate_b, gate_b]).astype(f32)
    ).reshape(128, 1)
    # eh' = h*64 + e ordering
    w1r = np.ascontiguousarray(
        w1.transpose(1, 2, 0).reshape(D, EH).astype(bf)
    ).reshape(KC, 128, EH)
    b1r = np.ascontiguousarray(b1.T.astype(f32)).reshape(MC, 128, 1)
    w2r = np.ascontiguousarray(
        w2.transpose(1, 0, 2).reshape(EH, D).astype(bf)
    ).reshape(MC, 128, D)
    common = {"gwr": gwr, "gbr": gbr, "w1r": w1r, "b1r": b1r, "w2r": w2r}
    return x_shards, common


def kernel(x, gate_w, gate_b, w1, b1, w2, _trace=False):
    import concourse.bass_utils as bass_utils

    x = np.asarray(x, dtype=np.float32)
    gate_w = np.asarray(gate_w, dtype=np.float32)
    gate_b = np.asarray(gate_b, dtype=np.float32)
    w1 = np.asarray(w1, dtype=np.float32)
    b1 = np.asarray(b1, dtype=np.float32)
    w2 = np.asarray(w2, dtype=np.float32)

    if "nc" not in _CACHE:
        _CACHE["nc"] = build(NTOK)
    nc = _CACHE["nc"]

    x_shards, common = host_prep(x, gate_w, gate_b, w1, b1, w2)
    in_maps = [dict(common, xT=x_shards[c]) for c in range(NCORES)]
    res = bass_utils.run_bass_kernel_spmd(
        nc, in_maps, core_ids=list(range(NCORES)), trace=_trace
    )
    _CACHE["last_results"] = res
    outs = [r["outT"].reshape(D, NTOK).T for r in res.results]
    return np.ascontiguousarray(np.concatenate(outs, axis=0), dtype=np.float32)


# revision 21
# speedup vs baseline: 1.1876x; 1.1876x over previous
"""MoE MLP (dense all-experts routing) Trainium2 Bass kernel.

Math (reference):
    g   = softmax(x @ gate_w + gate_b)            # [N, E]
    h   = relu(einsum("nd,edh->neh", x, w1) + b1) # [N, E, H]
    out = einsum("neh,ehd,ne->nd", h, w2, g)      # [N, D]

With E=64, H=16 (E*H = 1024 = D) this is two dense [1024,1024] matmuls plus a
small gate matmul.  We reorder the expert-hidden axis as  eh' = h*64 + e
(h-major).  Then the gating multiply  hg[n, eh'] = h[n, eh'] * g[n, eh' mod 64]
is a plain elementwise multiply of every 128-row tile of h^T by one shared
[128, tok] tile g2^T (gate probs duplicated twice along partitions) -- no
per-expert broadcast/expansion is needed anywhere.

Layouts on device are feature-major (x^T, h^T, out^T); tokens are the matmul
moving (free) dimension.  Sharding: data-parallel over tokens, 4096 per core,
8 cores, no collectives.  Matmuls run in bf16 with fp32 PSUM accumulation;
softmax runs in fp32 (no max-subtraction: logits here are bounded by ~+-4).
"""

import numpy as np
import ml_dtypes

N, D, E, H = 32768, 1024, 64, 16
EH = E * H  # 1024
NCORES = 8
NTOK = N // NCORES  # tokens per core
TT = 512            # token tile (one PSUM bank of fp32)
KC = D // 128       # 8 contraction chunks for D
MC = EH // 128      # 8 output row-tiles for EH (and for D in stage 2)

_CACHE = {}

# matmul input dtype for x / w1 / w2 / gate_w: "bf16" (fast, ~4.7e-3 rel err)
# or "f32r" (tf32-like PE mode, ~1e-4 rel err, same 1 cycle/row at N=512)
MM_DTYPE = "bf16"


def build(n_tok=NTOK, mm_dtype=None):
    """Build + compile the per-core Bass kernel for n_tok tokens."""
    import concourse.bass as bass
    import concourse.mybir as mybir
    import concourse.tile as tile
    from concourse import bacc

    f32 = mybir.dt.float32
    bf16 = mybir.dt.bfloat16
    mmdt = bf16 if (mm_dtype or MM_DTYPE) == "bf16" else mybir.dt.float32r
    AF = mybir.ActivationFunctionType
    nt = n_tok // TT
    assert n_tok % TT == 0

    nc = bacc.Bacc("TRN2", target_bir_lowering=False, debug=False)

    xT = nc.dram_tensor("xT", [KC, 128, n_tok], mmdt, kind="ExternalInput")
    gwr = nc.dram_tensor("gwr", [KC, 128, 128], mmdt, kind="ExternalInput")
    gbr = nc.dram_tensor("gbr", [128, 1], f32, kind="ExternalInput")
    w1r = nc.dram_tensor("w1r", [KC, 128, EH], mmdt, kind="ExternalInput")
    b1r = nc.dram_tensor("b1r", [MC, 128, 1], f32, kind="ExternalInput")
    w2r = nc.dram_tensor("w2r", [MC, 128, D], mmdt, kind="ExternalInput")
    outT = nc.dram_tensor("outT", [MC, 128, n_tok], f32, kind="ExternalOutput")

    with tile.TileContext(nc) as tc:
        with (
            tc.tile_pool(name="consts", bufs=1) as consts,
            tc.tile_pool(name="xp", bufs=2) as xp,
            tc.tile_pool(name="sp", bufs=2) as sp,
            tc.tile_pool(name="hp", bufs=3) as hp,
            tc.tile_pool(name="ps1", bufs=1, space=bass.MemorySpace.PSUM) as ps1,
            tc.tile_pool(name="ps2", bufs=3, space=bass.MemorySpace.PSUM) as ps2,
            tc.tile_pool(name="ps3", bufs=3, space=bass.MemorySpace.PSUM) as ps3,
        ):
            # --- resident weights (gate first, w2 last: needed latest;
            # on the gpsimd queue so the sync queue is dedicated to x tiles) ---
            gw_sb = consts.tile([128, KC, 128], mmdt)
            nc.gpsimd.dma_start(out=gw_sb[:], in_=gwr[:].rearrange("k p m -> p k m"))
            gb_sb = consts.tile([128, 1], f32)
            nc.gpsimd.dma_start(out=gb_sb[:], in_=gbr[:])
            b1_sb = consts.tile([128, MC, 1], f32)
            nc.gpsimd.dma_start(out=b1_sb[:], in_=b1r[:].rearrange("m p o -> p m o"))
            # w1 first (needed first), w2 after — gpsimd queue is FIFO so w1
            # gets the full DMA bandwidth before w2 starts
            w1_sb = consts.tile([128, KC, EH], mmdt)
            nc.gpsimd.dma_start(out=w1_sb[:], in_=w1r[:].rearrange("k p m -> p k m"))
            w2_sb = consts.tile([128, MC, D], mmdt)
            nc.gpsimd.dma_start(out=w2_sb[:], in_=w2r[:].rearrange("k p m -> p k m"))
            # dtypes: tensors consumed by a matmul must carry the matmul
            # dtype at their producer (BIR verifier: "rounded to FP32r");
            # DVE-only tensors stay f32 in f32r mode
            is_r = mmdt != bf16
            e2dt = mmdt          # consumed by the Z matmul
            hgdt = mmdt          # consumed by stage-2 matmuls
            vdt = f32 if is_r else bf16   # DVE-only tensors
            onesZ = consts.tile([64, 128], mmdt)
            if is_r:
                onesZf = consts.tile([64, 128], f32)
                nc.vector.memset(onesZf[:], 1.0)
                nc.vector.tensor_copy(onesZ[:], onesZf[:])
            else:
                nc.vector.memset(onesZ[:], 1.0)

            for t in range(nt):
                ts = slice(t * TT, (t + 1) * TT)
                xt = xp.tile([128, KC, TT], mmdt)
                for k in range(KC):
                    nc.sync.dma_start(out=xt[:, k, :], in_=xT[k, :, ts])

                # --- gate logits (both 64-expert halves identical) ---
                lg = ps1.tile([128, TT], f32, tag="lg")
                for k in range(KC):
                    nc.tensor.matmul(
                        lg[:], gw_sb[:, k, :], xt[:, k, :],
                        start=(k == 0), stop=(k == KC - 1),
                    )
                # e2 = exp(logits + gate_b)
                e2 = sp.tile([128, TT], e2dt, tag="e2")
                nc.scalar.activation(e2[:], lg[:], AF.Exp, bias=gb_sb[:], scale=1.0)

                hg = sp.tile([128, MC, TT], hgdt, tag="hg")
                h_tiles = []

                def stage1(m):
                    hps = ps2.tile([128, TT], f32, tag="hps")
                    for k in range(KC):
                        nc.tensor.matmul(
                            hps[:], w1_sb[:, k, m * 128:(m + 1) * 128], xt[:, k, :],
                            start=(k == 0), stop=(k == KC - 1),
                        )
                    h = hp.tile([128, TT], vdt, tag="h")
                    nc.scalar.activation(
                        h[:], hps[:], AF.Relu, bias=b1_sb[:, m, :], scale=1.0
                    )
                    h_tiles.append((m, h))

                # run two stage-1 tiles before the softmax matmuls so the PE
                # never stalls on ACT/DVE latency
                stage1(0)
                stage1(1)

                # Z broadcast to all 128 partitions via ones-matmul, then
                # g2 = e2 * (1/Z)
                zb = ps1.tile([128, TT], f32, tag="zb")
                nc.tensor.matmul(zb[:], onesZ[:], e2[0:64, :], start=True, stop=True)
                rzb = sp.tile([128, TT], f32, tag="rzb")
                nc.vector.reciprocal_approx_fast(rzb[:], zb[:])
                g2 = sp.tile([128, TT], vdt, tag="g2")
                nc.vector.tensor_mul(g2[:], e2[:].bitcast(f32) if is_r else e2[:], rzb[:])

                for m, h in h_tiles:
                    nc.vector.tensor_mul(hg[:, m, :], h[:], g2[:])
                for m in range(2, MC):
                    stage1(m)
                    _, h = h_tiles[-1]
                    nc.vector.tensor_mul(hg[:, m, :], h[:], g2[:])

                # --- stage 2: out^T tiles ---
                for m2 in range(MC):
                    ops = ps3.tile([128, TT], f32, tag="ops")
                    for k in range(MC):
                        nc.tensor.matmul(
                            ops[:], w2_sb[:, k, m2 * 128:(m2 + 1) * 128],
                            hg[:, k, :],
                            start=(k == 0), stop=(k == MC - 1),
                        )
                    osb = hp.tile([128, TT], f32, tag="osb")
                    if m2 % 2 == 0:
                        nc.scalar.copy(osb[:], ops[:])
                    else:
                        nc.vector.tensor_copy(osb[:], ops[:])
                    nc.gpsimd.dma_start(out=outT[m2, :, ts], in_=osb[:])

    nc.compile()
    return nc


def host_prep(x, gate_w, gate_b, w1, b1, w2):
    bf = ml_dtypes.bfloat16 if MM_DTYPE == "bf16" else np.float32
    f32 = np.float32
    xT = np.ascontiguousarray(x.astype(bf).T).reshape(KC, 128, N)
    x_shards = [
        np.ascontiguousarray(xT[:, :, c * NTOK:(c + 1) * NTOK]) for c in range(NCORES)
    ]
    gwr = np.ascontiguousarray(
        np.concatenate([gate_w, gate_w], axis=1).astype(bf)
    ).reshape(KC, 128, 128)
    gbr = np.ascontiguousarray(
        np.concatenate([gate_b, gate_b]).astype(f32)
    ).reshape(128, 1)
    # eh' = h*64 + e ordering
    w1r = np.ascontiguousarray(
        w1.transpose(1, 2, 0).reshape(D, EH).astype(bf)
    ).reshape(KC, 128, EH)
    b1r = np.ascontiguousarray(b1.T.astype(f32)).reshape(MC, 128, 1)
    w2r = np.ascontiguousarray(
        w2.transpose(1, 0, 2).reshape(EH, D).astype(bf)
    ).reshape(MC, 128, D)
    common = {"gwr": gwr, "gbr": gbr, "w1r": w1r, "b1r": b1r, "w2r": w2r}
    return x_shards, common


def kernel(x, gate_w, gate_b, w1, b1, w2, _trace=False):
    import concourse.bass_utils as bass_utils

    x = np.asarray(x, dtype=np.float32)
    gate_w = np.asarray(gate_w, dtype=np.float32)
    gate_b = np.asarray(gate_b, dtype=np.float32)
    w1 = np.asarray(w1, dtype=np.float32)
    b1 = np.asarray(b1, dtype=np.float32)
    w2 = np.asarray(w2, dtype=np.float32)

    if "nc" not in _CACHE:
        _CACHE["nc"] = build(NTOK)
    nc = _CACHE["nc"]

    x_shards, common = host_prep(x, gate_w, gate_b, w1, b1, w2)
    in_maps = [dict(common, xT=x_shards[c]) for c in range(NCORES)]
    res = bass_utils.run_bass_kernel_spmd(
        nc, in_maps, core_ids=list(range(NCORES)), trace=_trace
    )
    _CACHE["last_results"] = res
    outs = [r["outT"].reshape(D, NTOK).T for r in res.results]
    return np.ascontiguousarray(np.concatenate(outs, axis=0), dtype=np.float32)
